# revision 7
# baseline (speedup 1.0000x reference)
"""CRF negative log-likelihood kernel for Trainium2 (8 NeuronCores).

B=256, S=512, T=128. Data-parallel over batch: 32 sequences per core.

Algorithm (per core):
  - Partition function via the forward algorithm in exp-space:
      alpha_t = (E^T alpha_{t-1}) . x_t,  E = exp(transitions),
      x_t = exp(emissions_t - C_BIAS).
  - Time-segmented evaluation: the 511-step product of positive transfer
    matrices is split into 64 segments of ~8 steps.  Products of positive
    matrices contract to rank-1 extremely fast (verified: rank-1 junction
    error ~1e-13 for length-15 segments on this data), so
      Z = eEnd^T T_63 ... T_1 a_0 ~= (eEnd.f_63) prod_s sum(f_s) / 128^63,
    where f_s = T_s 1 is a forward probe through segment s and a_0 is the
    true prefix chain.  All 64 segment chains advance in parallel, giving
    8 sequential steps instead of 511 (measured end-to-end rel err 9e-5).
  - Per global step, two pipeline groups of 32 segments each run
    [128x128]x[128x512] matmuls (shared stationary E, no weight swaps)
    and one fused DVE multiply (PSUM . x -> bf16 state).
  - Gold path score: emit = ones-matmul reduce of (one-hot . emissions),
    with the product split between gpsimd (fp8 chunks, overlapping the
    chains) and the vector engine (bf16 tail chunks); transition sum via
    host-built per-sequence pair-count matrix; start/end via tiny one-hot
    matmuls (packed into spare PSUM rows of the emit bank).
  - Output nll[b] = logZ[b] - score[b].

Host prep is index manipulation / dtype / layout permutation only.
"""

import numpy as np
import ml_dtypes

bf16 = ml_dtypes.bfloat16
fp8 = ml_dtypes.float8_e4m3fn

B, S, T = 256, 512, 128
NCORES = 8
BS = B // NCORES            # 32
C_BIAS = 5.8
NSEG = 64                   # time segments (= parallel chains)
NSTEP = 8                   # sequential steps per segment
GW = 32 * BS                # group width: 32 segments x 32 seqs = 1024
CHAIN = NSTEP * 2 * GW      # 16384 chain columns
T0OFF = CHAIN               # t=0 block at the end
NCOL = CHAIN + BS           # 16416
NCH = 8                     # 2048-col chunks for DMA/compute overlap
CH = CHAIN // NCH           # 2048
KADD = float(S * C_BIAS - (NSEG - 1) * np.log(T))

# emit one-hot product split: gpsimd does fp8 chunks 0-4 (+ t0 block)
# overlapping the chain phase, the vector engine takes bf16 chunks 5-7
# in the tail.
GP_CHUNKS = (0, 1, 2, 3, 4)
DVE_CHUNKS = (5, 6, 7)
FP8_CHUNKS = set(GP_CHUNKS)

_CACHED = {}


def _build_bass():
    from contextlib import ExitStack
    import concourse.bacc as bacc
    import concourse.tile as tile
    from concourse import mybir

    f32 = mybir.dt.float32
    bft = mybir.dt.bfloat16
    f8 = mybir.dt.float8e4
    ALU = mybir.AluOpType
    ACTF = mybir.ActivationFunctionType

    def cdt(c):
        return f8 if c in FP8_CHUNKS else bft

    nc = bacc.Bacc("TRN2", target_bir_lowering=False, debug=False)

    # ---- DRAM I/O (per-core shapes) ----
    em_ds = [nc.dram_tensor(f"em{c}", [T, CH], cdt(c), kind="ExternalInput")
             for c in range(NCH)]
    oh_ds = [nc.dram_tensor(f"oh{c}", [T, CH], cdt(c), kind="ExternalInput")
             for c in range(NCH)]
    em0_d = nc.dram_tensor("em_t0", [T, BS], f8, kind="ExternalInput")
    oh0_d = nc.dram_tensor("oh_t0", [T, BS], f8, kind="ExternalInput")
    cm_d = nc.dram_tensor("cm", [T, T * BS], bft, kind="ExternalInput")
    trb_d = nc.dram_tensor("trb", [T, T * BS], bft, kind="ExternalInput")
    trf_d = nc.dram_tensor("trf", [T, T], f32, kind="ExternalInput")
    stf_d = nc.dram_tensor("stf", [T, 1], f32, kind="ExternalInput")
    enf_d = nc.dram_tensor("enf", [T, 1], f32, kind="ExternalInput")
    stb_d = nc.dram_tensor("stb", [T, 1], bft, kind="ExternalInput")
    enb_d = nc.dram_tensor("enb", [T, 1], bft, kind="ExternalInput")
    out_d = nc.dram_tensor("out", [1, BS], f32, kind="ExternalOutput")

    with tile.TileContext(nc) as tc, ExitStack() as ctx:
        big = ctx.enter_context(tc.tile_pool(name="big", bufs=1))
        small = ctx.enter_context(tc.tile_pool(name="small", bufs=1))
        wpa = ctx.enter_context(tc.tile_pool(name="wa", bufs=2))
        wpb = ctx.enter_context(tc.tile_pool(name="wb", bufs=3))
        ppool = ctx.enter_context(tc.tile_pool(name="p1", bufs=1, space="PSUM"))

        # ---- big SBUF ----
        emc = [big.tile([T, CH], cdt(c), tag=f"em{c}", name=f"em{c}")
               for c in range(NCH)]
        ohc = [big.tile([T, CH], cdt(c), tag=f"oh{c}", name=f"oh{c}")
               for c in range(NCH)]
        xc = [big.tile([T, CH], bft, tag=f"x{c}", name=f"x{c}")
              for c in range(NCH)]
        mskc = [big.tile([T, CH], bft, tag=f"msk{c}", name=f"msk{c}")
                for c in range(NCH)]
        em_t0 = big.tile([T, BS], f8, tag="em_t0")
        oh_t0 = big.tile([T, BS], f8, tag="oh_t0")
        x_t0 = big.tile([T, BS], bft, tag="x_t0")
        msk_t0 = big.tile([T, BS], bft, tag="msk_t0")
        cm = big.tile([T, T * BS], bft, tag="cm")
        trb = big.tile([T, T * BS], bft, tag="trb")
        mtr = big.tile([T, T * BS], bft, tag="mtr")

        # ---- small SBUF ----
        E_sb = small.tile([T, T], bft, tag="E")
        tr_raw = small.tile([T, T], f32, tag="tr_raw")
        ones_cb = small.tile([T, 1], bft, tag="ones_cb")
        st_f = small.tile([T, 1], f32, tag="st_f")
        en_f = small.tile([T, 1], f32, tag="en_f")
        st_b = small.tile([T, 1], bft, tag="st_b")
        en_b = small.tile([T, 1], bft, tag="en_b")
        exp_st = small.tile([T, 1], f32, tag="exp_st")
        exp_en_b = small.tile([T, 1], bft, tag="exp_en_b")
        nbias = small.tile([T, 1], f32, tag="nbias")
        lnwarm = small.tile([1, 1], f32, tag="lnwarm")
        lnv = small.tile([1, 2 * GW], bft, tag="lnv")
        red0 = small.tile([1, BS], f32, tag="red0")
        red1 = small.tile([1, BS], f32, tag="red1")
        red1b = small.tile([1, BS], f32, tag="red1b")
        red2 = small.tile([1, BS], f32, tag="red2")
        red2b = small.tile([1, BS], f32, tag="red2b")
        acc = small.tile([1, BS], f32, tag="acc")
        out_sb = small.tile([1, BS], f32, tag="out_sb")

        # ---- PSUM: vA(2 banks) + vB(2) + emit(1) + tran(1) = 6 banks ----
        vA = ppool.tile([T, GW], f32, tag="vA")
        vB = ppool.tile([T, GW], f32, tag="vB")
        emit_ps = ppool.tile([T, 16 * BS], f32, tag="emit_ps")
        tran_ps = ppool.tile([T, 16 * BS], f32, tag="tran_ps")

        # ================= DMA issue (4 parallel queues) =================
        # SP queue: chain-critical stream
        nc.sync.dma_start(out=tr_raw, in_=trf_d.ap())
        nc.sync.dma_start(out=emc[0], in_=em_ds[0].ap())
        nc.sync.dma_start(out=st_f, in_=stf_d.ap())
        nc.sync.dma_start(out=en_f, in_=enf_d.ap())
        nc.sync.dma_start(out=em_t0, in_=em0_d.ap())
        nc.sync.dma_start(out=emc[1], in_=em_ds[1].ap())
        nc.sync.dma_start(out=emc[2], in_=em_ds[2].ap())
        nc.sync.dma_start(out=emc[6], in_=em_ds[6].ap())
        nc.sync.dma_start(out=emc[7], in_=em_ds[7].ap())
        nc.sync.dma_start(out=st_b, in_=stb_d.ap())
        nc.sync.dma_start(out=en_b, in_=enb_d.ap())
        # ACT queue: mid emissions chunks (issued before the exp stream)
        nc.scalar.dma_start(out=emc[3], in_=em_ds[3].ap())
        nc.scalar.dma_start(out=emc[4], in_=em_ds[4].ap())
        nc.scalar.dma_start(out=emc[5], in_=em_ds[5].ap())
        # gpsimd queue: one-hot stream + count matrices (interleaved with
        # its products below)
        nc.gpsimd.dma_start(out=oh_t0, in_=oh0_d.ap())
        nc.gpsimd.dma_start(out=ohc[0], in_=oh_ds[0].ap())
        nc.gpsimd.dma_start(out=ohc[1], in_=oh_ds[1].ap())

        # ================= setup (ACT + DVE) =================
        nc.vector.memset(ones_cb, 1.0)
        nc.vector.memset(nbias, -C_BIAS)
        nc.vector.memset(lnwarm, 1.0)
        nc.scalar.activation(lnwarm, lnwarm, ACTF.Ln)   # preload Ln table
        nc.scalar.activation(E_sb, tr_raw, ACTF.Exp)
        nc.scalar.activation(exp_st, st_f, ACTF.Exp)
        nc.scalar.activation(exp_en_b, en_f, ACTF.Exp)
        nc.scalar.activation(x_t0, em_t0, ACTF.Exp, bias=nbias[:, :])
        for c in range(NCH):
            nc.scalar.activation(xc[c], emc[c], ACTF.Exp, bias=nbias[:, :])

        # chain states: probes start at 1.0; seg 0 carries the true prefix
        wA = wpa.tile([T, GW], bft, tag="wA")
        nc.vector.memset(wA, 1.0)
        nc.vector.tensor_scalar(out=wA[:, 0:BS], in0=x_t0[:, :],
                                scalar1=exp_st[:, :], scalar2=None, op0=ALU.mult)
        wB = wpb.tile([T, GW], bft, tag="wB")
        nc.vector.memset(wB, 1.0)

        # gpsimd: fp8 one-hot products, interleaved with its DMA issues
        gp_work = [(msk_t0, oh_t0, em_t0), (mskc[0], ohc[0], emc[0]),
                   (mskc[1], ohc[1], emc[1]), (mskc[2], ohc[2], emc[2]),
                   (mskc[3], ohc[3], emc[3]), (mskc[4], ohc[4], emc[4])]

        # ================= 8 global chain steps =================
        wB_prev = None
        for i in range(NSTEP):
            nc.tensor.matmul(vA[:, 0:512], lhsT=E_sb[:, :], rhs=wA[:, 0:512],
                             start=True, stop=True)
            nc.tensor.matmul(vA[:, 512:GW], lhsT=E_sb[:, :], rhs=wA[:, 512:GW],
                             start=True, stop=True)
            wA2 = wpa.tile([T, GW], bft, tag="wA")
            nc.vector.tensor_tensor(out=wA2, in0=vA[:, :],
                                    in1=xc[i][:, 0:GW], op=ALU.mult)
            wA = wA2
            nc.tensor.matmul(vB[:, 0:512], lhsT=E_sb[:, :], rhs=wB[:, 0:512],
                             start=True, stop=True)
            nc.tensor.matmul(vB[:, 512:GW], lhsT=E_sb[:, :], rhs=wB[:, 512:GW],
                             start=True, stop=True)
            wB2 = wpb.tile([T, GW], bft, tag="wB")
            nc.vector.tensor_tensor(out=wB2, in0=vB[:, :],
                                    in1=xc[i][:, GW:2 * GW], op=ALU.mult)
            if i == NSTEP - 2:
                wB_prev = wB2          # seg 63 final state (7 steps)
            wB = wB2
            # gpsimd one-hot products + remaining DMA issues
            if i == 0:
                o, a, b_ = gp_work[0]
                nc.gpsimd.tensor_tensor(out=o, in0=a, in1=b_, op=ALU.mult)
                o, a, b_ = gp_work[1]
                nc.gpsimd.tensor_tensor(out=o, in0=a, in1=b_, op=ALU.mult)
                nc.gpsimd.dma_start(out=ohc[2], in_=oh_ds[2].ap())
                nc.gpsimd.dma_start(out=ohc[3], in_=oh_ds[3].ap())
            elif i in (1, 2, 3, 4):
                o, a, b_ = gp_work[i + 1]
                nc.gpsimd.tensor_tensor(out=o, in0=a, in1=b_, op=ALU.mult)
                if i == 1:
                    nc.gpsimd.dma_start(out=ohc[4], in_=oh_ds[4].ap())
                    nc.gpsimd.dma_start(out=ohc[5], in_=oh_ds[5].ap())
                if i == 2:
                    nc.gpsimd.dma_start(out=ohc[6], in_=oh_ds[6].ap())
                    nc.gpsimd.dma_start(out=ohc[7], in_=oh_ds[7].ap())
                if i == 3:
                    nc.gpsimd.dma_start(out=cm, in_=cm_d.ap())
                    nc.gpsimd.dma_start(out=trb, in_=trb_d.ap())
            # emit reduce matmuls for chunks already produced (ample margin)
            if i in (5, 6, 7):
                cready = i - 5          # gpsimd chunk finished ~4us earlier
                for q in range(4):
                    kk = cready * 4 + q
                    nc.tensor.matmul(emit_ps[0:1, 0:512],
                                     lhsT=ones_cb[:, :],
                                     rhs=mskc[cready][:, q * 512:(q + 1) * 512],
                                     start=(kk == 0), stop=False,
                                     tile_position=(0, 0))

        # ================= segment stitching =================
        nc.tensor.matmul(vA[0:1, 0:512], lhsT=ones_cb[:, :], rhs=wA[:, 0:512],
                         start=True, stop=True)
        nc.tensor.matmul(vA[0:1, 512:GW], lhsT=ones_cb[:, :], rhs=wA[:, 512:GW],
                         start=True, stop=True)
        nc.tensor.matmul(vB[0:1, 0:512], lhsT=ones_cb[:, :], rhs=wB[:, 0:512],
                         start=True, stop=True)
        nc.tensor.matmul(vB[0:1, 512:992], lhsT=ones_cb[:, :], rhs=wB[:, 512:992],
                         start=True, stop=True)
        nc.tensor.matmul(vB[0:1, 992:GW], lhsT=exp_en_b[:, :],
                         rhs=wB_prev[:, 992:GW], start=True, stop=True)
        nc.scalar.activation(lnv[:, 0:GW], vA[0:1, :], ACTF.Ln)
        nc.scalar.activation(lnv[:, GW:2 * GW], vB[0:1, :], ACTF.Ln)
        lnv3 = lnv[:, :].rearrange("o (s b) -> o b s", b=BS)
        nc.vector.tensor_reduce(red0, lnv3, axis=mybir.AxisListType.X, op=ALU.add)

        # ================= gold-path score (tails) =================
        # vector engine: bf16 one-hot chunks + transition product (half)
        for c in DVE_CHUNKS:
            nc.vector.tensor_tensor(out=mskc[c], in0=ohc[c], in1=emc[c],
                                    op=ALU.mult)
        nc.vector.tensor_tensor(out=mtr[:, 0:CH], in0=cm[:, 0:CH],
                                in1=trb[:, 0:CH], op=ALU.mult)
        nc.gpsimd.tensor_tensor(out=mtr[:, CH:2 * CH], in0=cm[:, CH:2 * CH],
                                in1=trb[:, CH:2 * CH], op=ALU.mult)

        # start/end gold scores into spare emit_ps rows (64 / 96)
        nc.tensor.matmul(emit_ps[64:65, 0:BS], lhsT=st_b[:, :], rhs=oh_t0[:, :],
                         start=True, stop=True)
        nc.tensor.matmul(emit_ps[96:97, 0:BS], lhsT=en_b[:, :],
                         rhs=ohc[6][:, 2016:2048], start=True, stop=True,
                         tile_position=(0, 96))

        # remaining emit reduce matmuls: row 0 finishes chunks 3..4, row 32
        # takes the vector-engine chunks + t0 (accumulated last)
        for ci, c in enumerate((3, 4)):
            for q in range(4):
                nc.tensor.matmul(emit_ps[0:1, 0:512],
                                 lhsT=ones_cb[:, :],
                                 rhs=mskc[c][:, q * 512:(q + 1) * 512],
                                 start=False, stop=(ci == 1 and q == 3),
                                 tile_position=(0, 0))
        for ci, c in enumerate(DVE_CHUNKS):
            for q in range(4):
                nc.tensor.matmul(emit_ps[32:33, 0:512],
                                 lhsT=ones_cb[:, :],
                                 rhs=mskc[c][:, q * 512:(q + 1) * 512],
                                 start=(ci == 0 and q == 0), stop=False,
                                 tile_position=(0, 32))
        nc.tensor.matmul(emit_ps[32:33, 0:BS], lhsT=ones_cb[:, :],
                         rhs=msk_t0[:, :], start=False, stop=True,
                         tile_position=(0, 32))
        for q in range(8):
            g = q // 4
            nc.tensor.matmul(tran_ps[32 * g:32 * g + 1, :],
                             lhsT=ones_cb[:, :], rhs=mtr[:, q * 512:(q + 1) * 512],
                             start=(q % 4 == 0), stop=(q % 4 == 3),
                             tile_position=(0, 32 * g))

        # ================= final assembly =================
        emit3a = emit_ps[0:1, :].rearrange("o (t b) -> o b t", b=BS)
        emit3b = emit_ps[32:33, :].rearrange("o (t b) -> o b t", b=BS)
        nc.vector.tensor_reduce(red1, emit3a, axis=mybir.AxisListType.X, op=ALU.add)
        nc.vector.tensor_reduce(red1b, emit3b, axis=mybir.AxisListType.X, op=ALU.add)
        nc.vector.tensor_tensor(out=red1, in0=red1[:, :], in1=red1b[:, :], op=ALU.add)
        tran3a = tran_ps[0:1, :].rearrange("o (j b) -> o b j", b=BS)
        tran3b = tran_ps[32:33, :].rearrange("o (j b) -> o b j", b=BS)
        nc.vector.tensor_reduce(red2, tran3a, axis=mybir.AxisListType.X, op=ALU.add)
        nc.vector.tensor_reduce(red2b, tran3b, axis=mybir.AxisListType.X, op=ALU.add)
        nc.vector.tensor_tensor(out=red2, in0=red2[:, :], in1=red2b[:, :], op=ALU.add)
        nc.vector.tensor_scalar(out=acc, in0=red0, scalar1=KADD,
                                scalar2=None, op0=ALU.add)
        nc.vector.tensor_tensor(out=acc, in0=acc[:, :], in1=red1[:, :], op=ALU.subtract)
        nc.vector.tensor_tensor(out=acc, in0=acc[:, :], in1=red2[:, :], op=ALU.subtract)
        nc.vector.tensor_tensor(out=acc, in0=acc[:, :], in1=emit_ps[64:65, 0:BS],
                                op=ALU.subtract)
        nc.vector.tensor_tensor(out=out_sb, in0=acc[:, :], in1=emit_ps[96:97, 0:BS],
                                op=ALU.subtract)
        nc.sync.dma_start(out=out_d.ap(), in_=out_sb)

    nc.compile()
    return nc


def _host_prep(emissions, tags, transitions, start_transitions, end_transitions):
    """Per-core input maps. Index manipulation + dtype/layout prep only."""
    em_all = np.asarray(emissions, dtype=np.float32)
    tg_all = np.asarray(tags).astype(np.int64)
    trf = np.ascontiguousarray(np.asarray(transitions, np.float32))
    trb_full = np.ascontiguousarray(
        np.repeat(trf.astype(bf16)[:, :, None], BS, axis=2).reshape(T, T * BS))
    stf = np.asarray(start_transitions, np.float32).reshape(T, 1)
    enf = np.asarray(end_transitions, np.float32).reshape(T, 1)

    cols = np.arange(CHAIN)
    i_idx = cols >> 11
    rem = cols & 2047
    s_idx = (rem >> 10) * 32 + ((rem & 1023) >> 5)
    b_idx = cols & 31
    t_idx = 1 + NSTEP * s_idx + i_idx
    valid = t_idx <= S - 1
    tv = np.where(valid, t_idx, 0)

    in_maps = []
    for c in range(NCORES):
        emco = em_all[c * BS:(c + 1) * BS]            # [BS, S, T]
        tg = tg_all[c * BS:(c + 1) * BS]
        em_l = np.empty((T, CHAIN), dtype=np.float32)
        vals = emco[b_idx, tv, :]                     # [CHAIN, T]
        vals[~valid] = 0
        em_l[:, :] = vals.T
        oh_l = np.zeros((T, CHAIN), dtype=np.float32)
        tg_col = tg[b_idx, tv]
        oh_l[tg_col[valid], cols[valid]] = 1.0
        oh0 = np.zeros((T, BS), dtype=fp8)
        oh0[tg[:, 0], np.arange(BS)] = fp8(1.0)
        cmx = np.zeros((BS, T, T), dtype=np.float32)
        for b in range(BS):
            np.add.at(cmx[b], (tg[b, :-1], tg[b, 1:]), 1.0)
        cm_dev = np.ascontiguousarray(
            cmx.transpose(1, 2, 0).reshape(T, T * BS)).astype(bf16)
        mp = {
            "em_t0": np.ascontiguousarray(emco[:, 0, :].T).astype(fp8),
            "oh_t0": oh0, "cm": cm_dev, "trb": trb_full,
            "trf": trf, "stf": stf, "enf": enf,
            "stb": stf.astype(bf16), "enb": enf.astype(bf16),
        }
        for ch in range(NCH):
            dt = fp8 if ch in FP8_CHUNKS else bf16
            mp[f"em{ch}"] = np.ascontiguousarray(
                em_l[:, ch * CH:(ch + 1) * CH]).astype(dt)
            mp[f"oh{ch}"] = np.ascontiguousarray(
                oh_l[:, ch * CH:(ch + 1) * CH]).astype(dt)
        in_maps.append(mp)
    return in_maps


def _numpy_fallback(emissions, tags, mask, transitions, start_transitions,
                    end_transitions):
    em = np.asarray(emissions, np.float32)
    tg = np.asarray(tags).astype(np.int64)
    mk = np.asarray(mask).astype(np.float32)
    tr = np.asarray(transitions, np.float32)
    st = np.asarray(start_transitions, np.float32)
    en = np.asarray(end_transitions, np.float32)
    Bn, Sn, Tn = em.shape
    score = st[tg[:, 0]]
    emit = np.take_along_axis(em, tg[..., None], axis=2)[..., 0]
    score = score + (emit * mk).sum(1)
    score = score + (tr[tg[:, :-1], tg[:, 1:]] * mk[:, 1:]).sum(1)
    last = mk.astype(np.int64).sum(1) - 1
    score = score + en[np.take_along_axis(tg, last[:, None], 1)[:, 0]]
    fv = st[None, :] + em[:, 0]
    for t in range(1, Sn):
        m = fv.max(1, keepdims=True)
        fv = np.log(np.exp(fv - m) @ np.exp(tr)) + m + em[:, t]
    m = fv.max(1, keepdims=True)
    part = np.log((np.exp(fv - m) * np.exp(en)[None, :]).sum(1)) + m[:, 0]
    return -(score - part)


def kernel(emissions, tags, mask, transitions, start_transitions,
           end_transitions):
    em_arr = np.asarray(emissions)
    mask_arr = np.asarray(mask)
    tg_arr = np.asarray(tags).astype(np.int64)
    off_spec = (
        em_arr.shape != (B, S, T)
        or not mask_arr.all()
        or tg_arr.min() < 0 or tg_arr.max() >= T
    )
    if not off_spec:
        pair_counts = np.zeros((T * T,), np.int64)
        flat = tg_arr[:, :-1] * T + tg_arr[:, 1:]
        np.add.at(pair_counts, flat.reshape(-1), 1)
        if pair_counts.max() >= 256:
            per_b_max = 0
            for b in range(em_arr.shape[0]):
                cb = np.bincount(flat[b], minlength=T * T).max()
                per_b_max = max(per_b_max, cb)
            off_spec = per_b_max >= 256
    if off_spec:
        return _numpy_fallback(emissions, tags, mask, transitions,
                               start_transitions, end_transitions).astype(np.float32)

    from concourse import bass_utils

    if "nc" not in _CACHED:
        _CACHED["nc"] = _build_bass()
    nc = _CACHED["nc"]

    in_maps = _host_prep(emissions, tags, transitions, start_transitions,
                         end_transitions)
    res = bass_utils.run_bass_kernel_spmd(nc, in_maps, core_ids=list(range(NCORES)))
    out = np.concatenate([np.asarray(res.results[c]["out"]).reshape(BS)
                          for c in range(NCORES)])
    return out.astype(np.float32)
